# revision 5
# baseline (speedup 1.0000x reference)
"""Multi-head attention (B=2, S=2048, D=2048, H=16, Dh=128) on 8 TRN2 NeuronCores.

Tensor-parallel over heads: core c owns heads {2c, 2c+1}.

Per-core pipeline (bf16 data path, f32 PSUM/softmax):
  Phase A: QKV projection from replicated x^T. Q^T/K^T produced in
           [head_dim, token] layout (softmax scale folded into w_q on host);
           V natural [token, head_dim]. Batch 0 runs stand-alone; batch 1's
           projection is interleaved into the attention section below (one
           16-matmul group per attention super-slot) so the PE-only
           projection work fills the PE slack of the ScalarE-exp-bound
           attention phase.
  Phase B: attention per (local head, batch, 512-wide q tile), transposed
           formulation: S^T[k,q] tile pairs via K^T-stationary matmuls into a
           2-bank PSUM tile; one fused exp per pair on ScalarE straight out of
           PSUM (no max subtraction -- logits are N(0,1)-scaled). A continuous
           software pipeline across all q-tiles keeps the PE FIFO from ever
           blocking: PV^T accumulation trails exp by 2 super-slots, a VectorE
           add-tree reduces the P^T tiles to one [128,512] sum whose
           ones-vector matmul (into a borrowed PSUM bank) gives the softmax
           denominator, and the 1/l normalization (fast reciprocal + K=1
           outer-product broadcast + multiply) lands one q-tile behind.
  A2A:     per local head, TWO AllToAlls (token halves) move combined^T from
           head-sharded to token-sharded; the split halves the exposed
           latency of the last collective (phase C's odd partials start as
           soon as the first half lands). w_out^T rows are pre-permuted on
           host to match the (even heads | odd heads) order.
  Phase C: out-projection for the core's 512 tokens. Even-head partial sums
           (first head's A2A, long landed) run first, evicted to SBUF; odd
           partials are gated per token-half on the two head-1 collectives
           and combined on the VectorE, streaming the output DMA per tile.

Host: shards/transposes weights (bf16), replicates x^T, concatenates per-core
token slices into the full (2, 2048, 2048) float32 output.
"""

import sys

import ml_dtypes
import numpy as np

for _p in ("/opt/trn_rl_repo", "/root/.axon_site/_ro/trn_rl_repo"):
    if _p not in sys.path:
        sys.path.insert(0, _p)

from concourse import bacc, bass, mybir, tile
from concourse.bass_utils import run_bass_kernel_spmd

B = 2
S = 2048
D = 2048
H = 16
DH = 128
NC = 8
HL = 2  # heads per core
T = B * S  # 4096 tokens
TPC = T // NC  # 512 tokens per core
HTPC = TPC // 2  # 256: token half for the split A2A

F32 = mybir.dt.float32
F32R = mybir.dt.float32r
BF16 = mybir.dt.bfloat16
EXP = mybir.ActivationFunctionType.Exp

_graph_cache = {}


def build_graph(mm_dt=BF16):
    nc = bacc.Bacc(
        "TRN2",
        target_bir_lowering=False,
        debug=False,
        enable_asserts=False,
        num_devices=NC,
    )
    xT = nc.dram_tensor("xT", [D, T], BF16, kind="ExternalInput")
    ones_in = nc.dram_tensor("ones_in", [128, 1], BF16, kind="ExternalInput")
    ones_row_in = nc.dram_tensor("ones_row_in", [1, 128], BF16, kind="ExternalInput")
    wqkvT = nc.dram_tensor("wqkvT", [D, 3 * HL * DH], BF16, kind="ExternalInput")
    woutT = nc.dram_tensor("woutT", [D, D], BF16, kind="ExternalInput")
    out_ext = nc.dram_tensor("out", [TPC, D], F32, kind="ExternalOutput")

    DC = D // 128  # 16 contraction chunks of 128
    n_k = S // 128  # 16 k tiles per (b, head)
    n_p = n_k // 2  # 8 kt-pairs per q-tile
    n_qt = B * (S // 512)  # 8 q-tiles per head

    with tile.TileContext(nc) as tc:
        with (
            tc.tile_pool(name="constp", bufs=1) as constp,
            tc.tile_pool(name="dramp", bufs=1, space="DRAM") as dramp,
        ):
            ones_col = constp.tile([128, 1], BF16)
            nc.gpsimd.dma_start(out=ones_col[:], in_=ones_in.ap())
            ones_row = constp.tile([1, 128], BF16)
            nc.gpsimd.dma_start(out=ones_row[:], in_=ones_row_in.ap())

            # per (local head, token half) send/recv buffers for the split A2A
            a2a_send = [
                [
                    dramp.tile([NC, 128, HTPC], BF16, name=f"a2a_send{h}_{hf}")
                    for hf in range(2)
                ]
                for h in range(HL)
            ]
            a2a_recv = [
                [
                    dramp.tile([NC, 128, HTPC], BF16, name=f"a2a_recv{h}_{hf}")
                    for hf in range(2)
                ]
                for h in range(HL)
            ]

            with (
                tc.tile_pool(name="qkvp", bufs=1) as qkvp,
                tc.tile_pool(name="scrA", bufs=1) as scrA,
                tc.tile_pool(name="psA", bufs=2, space="PSUM") as psA,
            ):
                # persistent activations, split by batch so attention on b=0
                # only depends on the first half of the projection
                QT = [qkvp.tile([128, HL, S], mm_dt, name=f"QT{b}") for b in range(B)]
                KT = [qkvp.tile([128, HL, S], mm_dt, name=f"KT{b}") for b in range(B)]
                V = [
                    qkvp.tile([128, S // 128, HL * DH], mm_dt, name=f"V{b}")
                    for b in range(B)
                ]

                wqkv_s = scrA.tile([128, DC, 3 * HL * DH], mm_dt)
                # first chunks small so the first matmuls start ASAP
                for lo, hi in ((0, 1), (1, 2), (2, 4), (4, 8), (8, 12), (12, 16)):
                    nc.scalar.dma_start(
                        out=wqkv_s[:, lo:hi, :],
                        in_=wqkvT.ap()[lo * 128 : hi * 128, :].rearrange(
                            "(dc p) f -> p dc f", p=128
                        ),
                    )

                with (
                    tc.tile_pool(name="pC", bufs=1) as pC,
                    tc.tile_pool(name="evC", bufs=2) as evC,
                    tc.tile_pool(name="woutE", bufs=10) as woutE,
                    tc.tile_pool(name="xtp", bufs=8) as xtp,
                ):

                    def emit_xq(t, chunked=False):
                        """Load x^T token slice t as 4 quarter tiles of 4 chunks."""
                        xq = []
                        for qh in range(4):
                            xq_t = xtp.tile([128, 4, 512], mm_dt, tag="xq", name="xq")
                            eng = (nc.sync, nc.sync, nc.gpsimd, nc.gpsimd)[qh]
                            if qh == 1 and t >= 2:
                                eng = nc.scalar
                            if chunked:
                                for dcq in range(4):
                                    eng.dma_start(
                                        out=xq_t[:, dcq, :],
                                        in_=xT.ap()[
                                            qh * 512 + dcq * 128 : qh * 512
                                            + (dcq + 1) * 128,
                                            t * 512 : (t + 1) * 512,
                                        ].rearrange("(dc p) f -> p dc f", p=128)[
                                            :, 0, :
                                        ],
                                    )
                            else:
                                eng.dma_start(
                                    out=xq_t[:],
                                    in_=xT.ap()[
                                        qh * 512 : (qh + 1) * 512,
                                        t * 512 : (t + 1) * 512,
                                    ].rearrange("(dc p) f -> p dc f", p=128),
                                )
                            xq.append(xq_t)
                        return xq

                    def emit_a_group(t, gi, xq):
                        """One projection group: gi<4 -> Q/K column group gi for
                        512 tokens; gi>=4 -> V rows for token sub-tile gi-4."""
                        if gi < 4:
                            ps = psA.tile([128, 512], F32, tag="psA", name="psqk")
                            for dc in range(DC):
                                nc.tensor.matmul(
                                    ps[:],
                                    wqkv_s[:, dc, gi * 128 : (gi + 1) * 128],
                                    xq[dc // 4][:, dc % 4, :],
                                    start=(dc == 0),
                                    stop=(dc == DC - 1),
                                )
                            dest = QT if gi < HL else KT
                            hl = gi % HL
                            nc.vector.tensor_copy(
                                dest[t // 4][:, hl, (t % 4) * 512 : (t % 4 + 1) * 512],
                                ps[:],
                            )
                        else:
                            sub = gi - 4
                            psv = psA.tile([128, HL * DH], F32, tag="psA", name="psv")
                            for dc in range(DC):
                                nc.tensor.matmul(
                                    psv[:],
                                    xq[dc // 4][:, dc % 4, sub * 128 : (sub + 1) * 128],
                                    wqkv_s[:, dc, 2 * HL * DH : 3 * HL * DH],
                                    start=(dc == 0),
                                    stop=(dc == DC - 1),
                                )
                            nc.vector.tensor_copy(
                                V[t // 4][:, (t % 4) * 4 + sub, :], psv[:]
                            )

                    # ---------------- Phase A: projection of batch 0 ----------------
                    xq_tiles = {}
                    xq_tiles[0] = emit_xq(0, chunked=True)
                    xq_tiles[1] = emit_xq(1, chunked=True)
                    for t in range(4):
                        if t + 2 < 4:
                            xq_tiles[t + 2] = emit_xq(t + 2, chunked=True)
                        for gi in range(8):
                            emit_a_group(t, gi, xq_tiles[t])
                        del xq_tiles[t]

                    # -------- Phase B (attention + split A2A) with A(b=1)
                    # -------- merged into head 0's slots --------
                    weighth = {}
                    with (
                        tc.tile_pool(name="pB", bufs=2) as pB,
                        tc.tile_pool(name="psB", bufs=2, space="PSUM") as psB,
                    ):
                        pending = []

                        def flush_pending():
                            ps_o_p, rl_p, combT_p, qi_p, hl_p = pending.pop(0)
                            ps_b = psB.tile(
                                [128, 512], F32, tag="ps_s2", bufs=2, name="ps_b"
                            )
                            nc.tensor.matmul(
                                ps_b[:], ones_row[:], rl_p[:], start=True, stop=True
                            )
                            rlb = pB.tile([128, 512], BF16, tag="rlb")
                            nc.vector.tensor_copy(rlb[:], ps_b[:])
                            nc.vector.tensor_mul(
                                combT_p[:, qsl(qi_p)], ps_o_p[:], rlb[:]
                            )
                            # stream this q-tile's A2A shards (token halves)
                            for hf in range(2):
                                nc.sync.dma_start(
                                    out=a2a_send[hl_p][hf][qi_p],
                                    in_=combT_p[:, qsl_half(qi_p, hf)],
                                )

                        def qsl(qi):
                            b, qt = qi // 4, qi % 4
                            return slice(b * S + qt * 512, b * S + (qt + 1) * 512)

                        def qsl_half(qi, hf):
                            b, qt = qi // 4, qi % 4
                            lo = b * S + qt * 512 + hf * HTPC
                            return slice(lo, lo + HTPC)

                        def qsl_local(qi):
                            qt = qi % 4
                            return slice(qt * 512, (qt + 1) * 512)

                        NS = n_qt * n_p  # 64 super-slots per head
                        for hl in range(HL):
                            combT = pB.tile(
                                [128, T], BF16, tag="combT", name="combT", bufs=2
                            )
                            st = [None] * n_qt
                            if hl == 0:
                                xq_tiles[4] = emit_xq(4)
                                xq_tiles[5] = emit_xq(5)
                            if hl == 1:
                                # stream w_out^T even-head quarter-tiles during
                                # head 1 (DMA-idle window); consumption in
                                # phase C is g-major so bufs=10 windows cleanly.
                                for g in range(4):
                                    for e in range(4):
                                        wtile = woutE.tile(
                                            [128, 2, 512], BF16, tag="wout_e",
                                            name="wout_e",
                                        )
                                        nc.scalar.dma_start(
                                            out=wtile[:],
                                            in_=woutT.ap()[
                                                e * 256 : (e + 1) * 256,
                                                g * 512 : (g + 1) * 512,
                                            ].rearrange("(dc p) f -> p dc f", p=128),
                                        )
                                        weighth[(g, e)] = wtile
                            for s in range(NS + 16):
                                # ---- A(b=1) interleave: one group per slot
                                if hl == 0 and s < 32:
                                    t = 4 + s // 8
                                    if s % 8 == 0 and t + 2 < 8:
                                        xq_tiles[t + 2] = emit_xq(t + 2)
                                    emit_a_group(t, s % 8, xq_tiles[t])
                                    if s % 8 == 7:
                                        del xq_tiles[t]
                                # ---- S^T pair + fused exp
                                if s < NS:
                                    qi, pr = s // n_p, s % n_p
                                    b = qi // 4
                                    if pr == 0:
                                        st[qi] = {
                                            "ps_o": psB.tile(
                                                [128, 512], F32, tag="ps_o",
                                                name="ps_o",
                                            ),
                                            "pt2": [None] * n_p,
                                            "ptsums": [None] * n_p,
                                            "qsums": [None] * (n_p // 2),
                                            "hsums": [None] * 2,
                                        }
                                    ps_s = psB.tile(
                                        [128, 1024], F32, tag="ps_s2", bufs=2,
                                        name="ps_s",
                                    )
                                    for h in range(2):
                                        kt = 2 * pr + h
                                        nc.tensor.matmul(
                                            ps_s[:, h * 512 : (h + 1) * 512],
                                            KT[b][:, hl, kt * 128 : (kt + 1) * 128],
                                            QT[b][:, hl, qsl_local(qi)],
                                            start=True,
                                            stop=True,
                                        )
                                    pt2 = pB.tile(
                                        [128, 1024], mm_dt, tag="pt", bufs=4,
                                        name="pt2",
                                    )
                                    nc.scalar.activation(pt2[:], ps_s[:], EXP)
                                    st[qi]["pt2"][pr] = pt2
                                # ---- PV^T pair (trails by 2)
                                if 2 <= s < NS + 2:
                                    s2 = s - 2
                                    qi, pr = s2 // n_p, s2 % n_p
                                    b = qi // 4
                                    for h in range(2):
                                        kt = 2 * pr + h
                                        nc.tensor.matmul(
                                            st[qi]["ps_o"][:],
                                            V[b][:, kt, hl * DH : (hl + 1) * DH],
                                            st[qi]["pt2"][pr][
                                                :, h * 512 : (h + 1) * 512
                                            ],
                                            start=(kt == 0),
                                            stop=(kt == n_k - 1),
                                        )
                                # ---- DVE reduction tree for the denominator
                                if 2 <= s < NS + 2:
                                    gp = s - 2
                                    qi, j = gp // n_p, gp % n_p
                                    psm = pB.tile(
                                        [128, 512], mm_dt, tag="ptsum", bufs=5,
                                        name="psm",
                                    )
                                    nc.vector.tensor_add(
                                        psm[:],
                                        st[qi]["pt2"][j][:, 0:512],
                                        st[qi]["pt2"][j][:, 512:1024],
                                    )
                                    st[qi]["ptsums"][j] = psm
                                if 4 <= s < NS + 4 and s % 2 == 0:
                                    gq = (s - 4) // 2
                                    qi, j2 = gq // (n_p // 2), gq % (n_p // 2)
                                    qsm = pB.tile(
                                        [128, 512], mm_dt, tag="qsum", bufs=4,
                                        name="qsm",
                                    )
                                    nc.vector.tensor_add(
                                        qsm[:],
                                        st[qi]["ptsums"][2 * j2][:],
                                        st[qi]["ptsums"][2 * j2 + 1][:],
                                    )
                                    st[qi]["qsums"][j2] = qsm
                                if s >= 11 and (s - 11) % n_p == 0 and (s - 11) // n_p < n_qt:
                                    qi = (s - 11) // n_p
                                    hs = pB.tile(
                                        [128, 512], mm_dt, tag="hsum", bufs=2,
                                        name="hs0",
                                    )
                                    nc.vector.tensor_add(
                                        hs[:], st[qi]["qsums"][0][:], st[qi]["qsums"][1][:]
                                    )
                                    st[qi]["hsums"][0] = hs
                                if s >= 12 and (s - 12) % n_p == 0 and (s - 12) // n_p < n_qt:
                                    qi = (s - 12) // n_p
                                    hs = pB.tile(
                                        [128, 512], mm_dt, tag="hsum", bufs=2,
                                        name="hs1",
                                    )
                                    nc.vector.tensor_add(
                                        hs[:], st[qi]["qsums"][2][:], st[qi]["qsums"][3][:]
                                    )
                                    st[qi]["hsums"][1] = hs
                                if s >= 13 and (s - 13) % n_p == 0 and (s - 13) // n_p < n_qt:
                                    qi = (s - 13) // n_p
                                    osum = pB.tile(
                                        [128, 512], mm_dt, tag="osum", bufs=2,
                                        name="osum",
                                    )
                                    nc.vector.tensor_add(
                                        osum[:],
                                        st[qi]["hsums"][0][:],
                                        st[qi]["hsums"][1][:],
                                    )
                                    st[qi]["osum"] = osum
                                # ---- single denominator matmul + reciprocal
                                if s >= 14 and (s - 14) % n_p == 0 and (s - 14) // n_p < n_qt:
                                    qi = (s - 14) // n_p
                                    ps_l = psA.tile(
                                        [1, 512], F32, tag="psA", name="ps_l"
                                    )
                                    nc.tensor.matmul(
                                        ps_l[:],
                                        ones_col[:],
                                        st[qi]["osum"][:],
                                        start=True,
                                        stop=True,
                                    )
                                    rlf = pB.tile([1, 512], F32, tag="rlf", name="rlf")
                                    nc.vector.reciprocal_approx_fast(
                                        out=rlf[:], in_=ps_l[:]
                                    )
                                    rl = pB.tile([1, 512], BF16, tag="rl", name="rl")
                                    nc.vector.tensor_copy(rl[:], rlf[:])
                                    pending.append(
                                        (st[qi]["ps_o"], rl, combT, qi, hl)
                                    )
                                if s % n_p == 0 and s > 0 and pending:
                                    flush_pending()
                            # drain the pipeline before the send DMA reads combT
                            while pending:
                                flush_pending()
                            # shards were streamed per q-tile by the flushes;
                            # redistribute head->token sharding in two halves so
                            # phase C can start on the first half early.
                            for hf in range(2):
                                nc.gpsimd.collective_compute(
                                    "AllToAll",
                                    mybir.AluOpType.bypass,
                                    replica_groups=[list(range(NC))],
                                    ins=[a2a_send[hl][hf][:]],
                                    outs=[a2a_recv[hl][hf][:]],
                                )

                    # ---------------- Phase C: out projection ----------------
                    # pB/psB/xtp closed above: their SBUF backs woutO below.
                    with (
                        tc.tile_pool(name="woutO", bufs=16) as woutO,
                        tc.tile_pool(name="psC", bufs=2, space="PSUM") as psC,
                    ):
                        # odd-head w_out tiles: fully resident (wait-free DMAs),
                        # streamed during phase C's even half / A2A window.
                        for g in range(4):
                            for e in range(4, 8):
                                wtile = woutO.tile(
                                    [128, 2, 512], BF16, tag="wout_o", name="wout_o"
                                )
                                nc.scalar.dma_start(
                                    out=wtile[:],
                                    in_=woutT.ap()[
                                        e * 256 : (e + 1) * 256,
                                        g * 512 : (g + 1) * 512,
                                    ].rearrange("(dc p) f -> p dc f", p=128),
                                )
                                weighth[(g, e)] = wtile

                        # combined^T input tiles per (contraction chunk, half)
                        comb = {}
                        for hf in range(2):
                            for cc in range(DC):
                                hi, blk = (0, cc) if cc < 8 else (1, cc - 8)
                                ctile = pC.tile(
                                    [128, HTPC], BF16, tag="comb_in",
                                    name="comb_in", bufs=2 * DC,
                                )
                                nc.gpsimd.dma_start(
                                    out=ctile[:], in_=a2a_recv[hi][hf][blk]
                                )
                                comb[(cc, hf)] = ctile

                        def stat(cc, ts):
                            """[128,128] stationary: chunk cc, token sub-tile ts."""
                            return comb[(cc, ts // 2)][
                                :, (ts % 2) * 128 : (ts % 2 + 1) * 128
                            ]

                        # even-head partial sums first (head-0 A2A data, long
                        # landed), evicted to SBUF bf16.
                        partials = {}
                        for g in range(4):
                            for ts in range(TPC // 128):
                                psE = psC.tile([128, 512], F32, tag="psC")
                                for cc in range(8):
                                    nc.tensor.matmul(
                                        psE[:],
                                        stat(cc, ts),
                                        weighth[(g, cc // 2)][:, cc % 2, :],
                                        start=(cc == 0),
                                        stop=(cc == 7),
                                    )
                                pev = evC.tile(
                                    [128, 512], BF16, tag="pev", bufs=16, name="pev"
                                )
                                nc.scalar.copy(pev[:], psE[:])
                                partials[(g, ts)] = pev
                        # odd partials: token-half major so each group is gated
                        # only on the matching head-1 half-A2A.
                        for hf in range(2):
                            for g in range(4):
                                for ts in (2 * hf, 2 * hf + 1):
                                    psO = psC.tile([128, 512], F32, tag="psC")
                                    for cc in range(8, DC):
                                        nc.tensor.matmul(
                                            psO[:],
                                            stat(cc, ts),
                                            weighth[(g, cc // 2)][:, cc % 2, :],
                                            start=(cc == 8),
                                            stop=(cc == DC - 1),
                                        )
                                    ev = evC.tile([128, 512], F32, tag="ev")
                                    nc.vector.tensor_add(
                                        ev[:], psO[:], partials[(g, ts)][:]
                                    )
                                    nc.sync.dma_start(
                                        out=out_ext.ap()[
                                            ts * 128 : (ts + 1) * 128,
                                            g * 512 : (g + 1) * 512,
                                        ],
                                        in_=ev[:],
                                    )
    nc.finalize()
    return nc


def prep_inputs(x, w_qkv, w_out):
    """Host-side sharding. Returns list of per-core input dicts."""
    x = np.asarray(x, dtype=np.float32)
    w_qkv = np.asarray(w_qkv, dtype=np.float32)
    w_out = np.asarray(w_out, dtype=np.float32)

    xT = np.ascontiguousarray(x.reshape(T, D).T).astype(ml_dtypes.bfloat16)

    # w_out^T with rows permuted to (even heads | odd heads)
    woutT = w_out.T  # [cin, dout], cin = h*DH + d
    perm = [2 * i for i in range(8)] + [2 * i + 1 for i in range(8)]
    woutT_bf = np.ascontiguousarray(
        np.concatenate([woutT[h * DH : (h + 1) * DH] for h in perm], axis=0)
    ).astype(ml_dtypes.bfloat16)

    scale = np.float32(1.0 / np.sqrt(DH))
    ones = np.ones((128, 1), dtype=ml_dtypes.bfloat16)
    in_maps = []
    for c in range(NC):
        h0 = HL * c
        wq = w_qkv[h0 * DH : (h0 + HL) * DH] * scale  # [256, D]
        wk = w_qkv[H * DH + h0 * DH : H * DH + (h0 + HL) * DH]
        wv = w_qkv[2 * H * DH + h0 * DH : 2 * H * DH + (h0 + HL) * DH]
        wqkvT = np.ascontiguousarray(np.concatenate([wq, wk, wv], axis=0).T).astype(
            ml_dtypes.bfloat16
        )
        in_maps.append(
            {
                "xT": xT,
                "wqkvT": wqkvT,
                "woutT": woutT_bf,
                "ones_in": ones,
                "ones_row_in": np.ones((1, 128), dtype=ml_dtypes.bfloat16),
            }
        )
    return in_maps


def run(x, w_qkv, w_out, mm_dt=BF16, trace=False, tmpdir=None):
    key = str(mm_dt)
    if key not in _graph_cache:
        _graph_cache[key] = build_graph(mm_dt)
    nc = _graph_cache[key]
    in_maps = prep_inputs(x, w_qkv, w_out)
    res = run_bass_kernel_spmd(
        nc, in_maps, core_ids=list(range(NC)), trace=trace, tmpdir=tmpdir
    )
    out = np.concatenate([res.results[c]["out"] for c in range(NC)], axis=0)
    return out.reshape(B, S, D).astype(np.float32), res


def kernel(x, w_qkv, w_out):
    out, _ = run(x, w_qkv, w_out)
    return out


# revision 7
# speedup vs baseline: 1.0863x; 1.0863x over previous
"""Multi-head attention (B=2, S=2048, D=2048, H=16, Dh=128) on 8 TRN2 NeuronCores.

Tensor-parallel over heads: core c owns heads {2c, 2c+1}.

Per-core pipeline (bf16 data path, f32 PSUM/softmax):
  Phase A: QKV projection from replicated x^T. Q^T/K^T produced in
           [head_dim, token] layout (softmax scale folded into w_q on host);
           V natural [token, head_dim]. Batch 0 runs stand-alone; batch 1's
           projection is interleaved into the attention section below (one
           16-matmul group per attention super-slot) so the PE-only
           projection work fills the PE slack of the ScalarE-exp-bound
           attention phase.
  Phase B: attention per (local head, batch, 512-wide q tile), transposed
           formulation: S^T[k,q] tile pairs via K^T-stationary matmuls into a
           2-bank PSUM tile; one fused exp per pair on ScalarE straight out of
           PSUM (no max subtraction -- logits are N(0,1)-scaled). A continuous
           software pipeline across all q-tiles keeps the PE FIFO from ever
           blocking: PV^T accumulation trails exp by 2 super-slots, a VectorE
           add-tree reduces the P^T tiles to one [128,512] sum whose
           ones-vector matmul (into a borrowed PSUM bank) gives the softmax
           denominator, and the 1/l normalization (fast reciprocal + K=1
           outer-product broadcast + multiply) lands one q-tile behind.
  A2A:     per local head, TWO AllToAlls (token halves) move combined^T from
           head-sharded to token-sharded; the split halves the exposed
           latency of the last collective (phase C's odd partials start as
           soon as the first half lands). w_out^T rows are pre-permuted on
           host to match the (even heads | odd heads) order.
  Phase C: out-projection for the core's 512 tokens. Even-head partial sums
           (first head's A2A, long landed) run first, evicted to SBUF; odd
           partials are gated per token-half on the two head-1 collectives
           and combined on the VectorE, streaming the output DMA per tile.

Host: shards/transposes weights (bf16), replicates x^T, concatenates per-core
token slices into the full (2, 2048, 2048) float32 output.
"""

import sys

import ml_dtypes
import numpy as np

for _p in ("/opt/trn_rl_repo", "/root/.axon_site/_ro/trn_rl_repo"):
    if _p not in sys.path:
        sys.path.insert(0, _p)

from concourse import bacc, bass, mybir, tile
from concourse.bass_utils import run_bass_kernel_spmd

B = 2
S = 2048
D = 2048
H = 16
DH = 128
NC = 8
HL = 2  # heads per core
T = B * S  # 4096 tokens
TPC = T // NC  # 512 tokens per core
HTPC = TPC // 2  # 256: token half for the split A2A

F32 = mybir.dt.float32
F32R = mybir.dt.float32r
BF16 = mybir.dt.bfloat16
EXP = mybir.ActivationFunctionType.Exp

_graph_cache = {}


def build_graph(mm_dt=BF16):
    nc = bacc.Bacc(
        "TRN2",
        target_bir_lowering=False,
        debug=False,
        enable_asserts=False,
        num_devices=NC,
    )
    xT = nc.dram_tensor("xT", [D, T], BF16, kind="ExternalInput")
    ones_in = nc.dram_tensor("ones_in", [128, 1], BF16, kind="ExternalInput")
    ones_row_in = nc.dram_tensor("ones_row_in", [1, 128], BF16, kind="ExternalInput")
    wqkvT = nc.dram_tensor("wqkvT", [D, 3 * HL * DH], BF16, kind="ExternalInput")
    woutT = nc.dram_tensor("woutT", [D, D], BF16, kind="ExternalInput")
    out_ext = nc.dram_tensor("out", [TPC, D], F32, kind="ExternalOutput")

    DC = D // 128  # 16 contraction chunks of 128
    n_k = S // 128  # 16 k tiles per (b, head)
    n_p = n_k // 2  # 8 kt-pairs per q-tile
    n_qt = B * (S // 512)  # 8 q-tiles per head

    with tile.TileContext(nc) as tc:
        with (
            tc.tile_pool(name="constp", bufs=1) as constp,
            tc.tile_pool(name="dramp", bufs=1, space="DRAM") as dramp,
        ):
            ones_col = constp.tile([128, 1], BF16)
            nc.gpsimd.dma_start(out=ones_col[:], in_=ones_in.ap())
            ones_row = constp.tile([1, 128], BF16)
            nc.gpsimd.dma_start(out=ones_row[:], in_=ones_row_in.ap())

            # per (local head, token half) send/recv buffers for the split A2A
            a2a_send = [
                [
                    dramp.tile([NC, 128, HTPC], BF16, name=f"a2a_send{h}_{hf}")
                    for hf in range(2)
                ]
                for h in range(HL)
            ]
            a2a_recv = [
                [
                    dramp.tile([NC, 128, HTPC], BF16, name=f"a2a_recv{h}_{hf}")
                    for hf in range(2)
                ]
                for h in range(HL)
            ]

            with (
                tc.tile_pool(name="qkvp", bufs=1) as qkvp,
                tc.tile_pool(name="scrA", bufs=1) as scrA,
                tc.tile_pool(name="psA", bufs=2, space="PSUM") as psA,
            ):
                # persistent activations, split by batch so attention on b=0
                # only depends on the first half of the projection
                QT = [qkvp.tile([128, HL, S], mm_dt, name=f"QT{b}") for b in range(B)]
                KT = [qkvp.tile([128, HL, S], mm_dt, name=f"KT{b}") for b in range(B)]
                V = [
                    qkvp.tile([128, S // 128, HL * DH], mm_dt, name=f"V{b}")
                    for b in range(B)
                ]

                wqkv_s = scrA.tile([128, DC, 3 * HL * DH], mm_dt)
                # first chunks small so the first matmuls start ASAP
                for lo, hi in ((0, 1), (1, 2), (2, 4), (4, 8), (8, 12), (12, 16)):
                    nc.scalar.dma_start(
                        out=wqkv_s[:, lo:hi, :],
                        in_=wqkvT.ap()[lo * 128 : hi * 128, :].rearrange(
                            "(dc p) f -> p dc f", p=128
                        ),
                    )

                with (
                    tc.tile_pool(name="pC", bufs=1) as pC,
                    tc.tile_pool(name="evC", bufs=2) as evC,
                    tc.tile_pool(name="woutE", bufs=10) as woutE,
                    tc.tile_pool(name="xtp", bufs=8) as xtp,
                ):

                    def emit_xq(t, chunked=False):
                        """Load x^T token slice t as 4 quarter tiles of 4 chunks."""
                        xq = []
                        for qh in range(4):
                            xq_t = xtp.tile([128, 4, 512], mm_dt, tag="xq", name="xq")
                            eng = (nc.sync, nc.sync, nc.gpsimd, nc.gpsimd)[qh]
                            if qh == 1 and t >= 2:
                                eng = nc.scalar
                            if chunked:
                                for dcq in range(4):
                                    eng.dma_start(
                                        out=xq_t[:, dcq, :],
                                        in_=xT.ap()[
                                            qh * 512 + dcq * 128 : qh * 512
                                            + (dcq + 1) * 128,
                                            t * 512 : (t + 1) * 512,
                                        ].rearrange("(dc p) f -> p dc f", p=128)[
                                            :, 0, :
                                        ],
                                    )
                            else:
                                eng.dma_start(
                                    out=xq_t[:],
                                    in_=xT.ap()[
                                        qh * 512 : (qh + 1) * 512,
                                        t * 512 : (t + 1) * 512,
                                    ].rearrange("(dc p) f -> p dc f", p=128),
                                )
                            xq.append(xq_t)
                        return xq

                    def emit_a_group(t, gi, xq):
                        """One projection group: gi<4 -> Q/K column group gi for
                        512 tokens; gi>=4 -> V rows for token sub-tile gi-4."""
                        if gi < 4:
                            ps = psA.tile([128, 512], F32, tag="psA", name="psqk")
                            for dc in range(DC):
                                nc.tensor.matmul(
                                    ps[:],
                                    wqkv_s[:, dc, gi * 128 : (gi + 1) * 128],
                                    xq[dc // 4][:, dc % 4, :],
                                    start=(dc == 0),
                                    stop=(dc == DC - 1),
                                )
                            dest = QT if gi < HL else KT
                            hl = gi % HL
                            nc.vector.tensor_copy(
                                dest[t // 4][:, hl, (t % 4) * 512 : (t % 4 + 1) * 512],
                                ps[:],
                            )
                        else:
                            sub = gi - 4
                            psv = psA.tile([128, HL * DH], F32, tag="psA", name="psv")
                            for dc in range(DC):
                                nc.tensor.matmul(
                                    psv[:],
                                    xq[dc // 4][:, dc % 4, sub * 128 : (sub + 1) * 128],
                                    wqkv_s[:, dc, 2 * HL * DH : 3 * HL * DH],
                                    start=(dc == 0),
                                    stop=(dc == DC - 1),
                                )
                            nc.vector.tensor_copy(
                                V[t // 4][:, (t % 4) * 4 + sub, :], psv[:]
                            )

                    # ---------------- Phase A: projection of batch 0 ----------------
                    xq_tiles = {}
                    xq_tiles[0] = emit_xq(0, chunked=True)
                    xq_tiles[1] = emit_xq(1, chunked=True)
                    for t in range(4):
                        if t + 2 < 4:
                            xq_tiles[t + 2] = emit_xq(t + 2, chunked=True)
                        for gi in range(8):
                            emit_a_group(t, gi, xq_tiles[t])
                        del xq_tiles[t]

                    # -------- Phase B (attention + split A2A) with A(b=1)
                    # -------- merged into head 0's slots --------
                    weighth = {}
                    with (
                        tc.tile_pool(name="pB", bufs=2) as pB,
                        tc.tile_pool(name="psB", bufs=2, space="PSUM") as psB,
                    ):
                        pending = []

                        def flush_pending():
                            ps_o_p, rl_p, combT_p, qi_p, hl_p = pending.pop(0)
                            ps_b = psB.tile(
                                [128, 512], F32, tag="ps_s2", bufs=2, name="ps_b"
                            )
                            nc.tensor.matmul(
                                ps_b[:], ones_row[:], rl_p[:], start=True, stop=True
                            )
                            rlb = pB.tile([128, 512], BF16, tag="rlb")
                            nc.vector.tensor_copy(rlb[:], ps_b[:])
                            nc.vector.tensor_mul(
                                combT_p[:, qsl(qi_p)], ps_o_p[:], rlb[:]
                            )
                            # stream this q-tile's A2A shards (token halves)
                            for hf in range(2):
                                nc.sync.dma_start(
                                    out=a2a_send[hl_p][hf][qi_p],
                                    in_=combT_p[:, qsl_half(qi_p, hf)],
                                )

                        def qsl(qi):
                            b, qt = qi // 4, qi % 4
                            return slice(b * S + qt * 512, b * S + (qt + 1) * 512)

                        def qsl_half(qi, hf):
                            b, qt = qi // 4, qi % 4
                            lo = b * S + qt * 512 + hf * HTPC
                            return slice(lo, lo + HTPC)

                        def qsl_local(qi):
                            qt = qi % 4
                            return slice(qt * 512, (qt + 1) * 512)

                        NS = n_qt * n_p  # 64 super-slots per head
                        for hl in range(HL):
                            combT = pB.tile(
                                [128, T], BF16, tag="combT", name="combT", bufs=2
                            )
                            st = [None] * n_qt
                            if hl == 0:
                                xq_tiles[4] = emit_xq(4)
                                xq_tiles[5] = emit_xq(5)
                            for s in range(NS + 16):
                                # stream w_out^T even-head quarter-tiles during
                                # head 0's PE-rich slots (ScalarE idle there;
                                # in B1 the exp stream saturates ScalarE).
                                # Consumption in phase C is g-major so bufs=10
                                # windows cleanly.
                                if hl == 0 and 16 <= s < 32:
                                    ge = s - 16
                                    g, e = ge // 4, ge % 4
                                    wtile = woutE.tile(
                                        [128, 2, 512], BF16, tag="wout_e",
                                        name="wout_e",
                                    )
                                    nc.scalar.dma_start(
                                        out=wtile[:],
                                        in_=woutT.ap()[
                                            e * 256 : (e + 1) * 256,
                                            g * 512 : (g + 1) * 512,
                                        ].rearrange("(dc p) f -> p dc f", p=128),
                                    )
                                    weighth[(g, e)] = wtile
                                # ---- A(b=1) interleave: one group per slot
                                if hl == 0 and s < 32:
                                    t = 4 + s // 8
                                    if s % 8 == 0 and t + 2 < 8:
                                        xq_tiles[t + 2] = emit_xq(t + 2)
                                    emit_a_group(t, s % 8, xq_tiles[t])
                                    if s % 8 == 7:
                                        del xq_tiles[t]
                                # ---- S^T pair + fused exp
                                if s < NS:
                                    qi, pr = s // n_p, s % n_p
                                    b = qi // 4
                                    if pr == 0:
                                        st[qi] = {
                                            "ps_o": psB.tile(
                                                [128, 512], F32, tag="ps_o",
                                                name="ps_o",
                                            ),
                                            "pt2": [None] * n_p,
                                            "ptsums": [None] * n_p,
                                            "qsums": [None] * (n_p // 2),
                                            "hsums": [None] * 2,
                                        }
                                    ps_s = psB.tile(
                                        [128, 1024], F32, tag="ps_s2", bufs=2,
                                        name="ps_s",
                                    )
                                    for h in range(2):
                                        kt = 2 * pr + h
                                        nc.tensor.matmul(
                                            ps_s[:, h * 512 : (h + 1) * 512],
                                            KT[b][:, hl, kt * 128 : (kt + 1) * 128],
                                            QT[b][:, hl, qsl_local(qi)],
                                            start=True,
                                            stop=True,
                                        )
                                    pt2 = pB.tile(
                                        [128, 1024], mm_dt, tag="pt", bufs=4,
                                        name="pt2",
                                    )
                                    nc.scalar.activation(pt2[:], ps_s[:], EXP)
                                    st[qi]["pt2"][pr] = pt2
                                # ---- PV^T pair (trails by 2)
                                if 2 <= s < NS + 2:
                                    s2 = s - 2
                                    qi, pr = s2 // n_p, s2 % n_p
                                    b = qi // 4
                                    for h in range(2):
                                        kt = 2 * pr + h
                                        nc.tensor.matmul(
                                            st[qi]["ps_o"][:],
                                            V[b][:, kt, hl * DH : (hl + 1) * DH],
                                            st[qi]["pt2"][pr][
                                                :, h * 512 : (h + 1) * 512
                                            ],
                                            start=(kt == 0),
                                            stop=(kt == n_k - 1),
                                        )
                                # ---- DVE reduction tree for the denominator
                                if 2 <= s < NS + 2:
                                    gp = s - 2
                                    qi, j = gp // n_p, gp % n_p
                                    psm = pB.tile(
                                        [128, 512], mm_dt, tag="ptsum", bufs=5,
                                        name="psm",
                                    )
                                    nc.vector.tensor_add(
                                        psm[:],
                                        st[qi]["pt2"][j][:, 0:512],
                                        st[qi]["pt2"][j][:, 512:1024],
                                    )
                                    st[qi]["ptsums"][j] = psm
                                if 4 <= s < NS + 4 and s % 2 == 0:
                                    gq = (s - 4) // 2
                                    qi, j2 = gq // (n_p // 2), gq % (n_p // 2)
                                    qsm = pB.tile(
                                        [128, 512], mm_dt, tag="qsum", bufs=4,
                                        name="qsm",
                                    )
                                    nc.vector.tensor_add(
                                        qsm[:],
                                        st[qi]["ptsums"][2 * j2][:],
                                        st[qi]["ptsums"][2 * j2 + 1][:],
                                    )
                                    st[qi]["qsums"][j2] = qsm
                                if s >= 11 and (s - 11) % n_p == 0 and (s - 11) // n_p < n_qt:
                                    qi = (s - 11) // n_p
                                    hs = pB.tile(
                                        [128, 512], mm_dt, tag="hsum", bufs=2,
                                        name="hs0",
                                    )
                                    nc.vector.tensor_add(
                                        hs[:], st[qi]["qsums"][0][:], st[qi]["qsums"][1][:]
                                    )
                                    st[qi]["hsums"][0] = hs
                                if s >= 12 and (s - 12) % n_p == 0 and (s - 12) // n_p < n_qt:
                                    qi = (s - 12) // n_p
                                    hs = pB.tile(
                                        [128, 512], mm_dt, tag="hsum", bufs=2,
                                        name="hs1",
                                    )
                                    nc.vector.tensor_add(
                                        hs[:], st[qi]["qsums"][2][:], st[qi]["qsums"][3][:]
                                    )
                                    st[qi]["hsums"][1] = hs
                                if s >= 13 and (s - 13) % n_p == 0 and (s - 13) // n_p < n_qt:
                                    qi = (s - 13) // n_p
                                    osum = pB.tile(
                                        [128, 512], mm_dt, tag="osum", bufs=2,
                                        name="osum",
                                    )
                                    nc.vector.tensor_add(
                                        osum[:],
                                        st[qi]["hsums"][0][:],
                                        st[qi]["hsums"][1][:],
                                    )
                                    st[qi]["osum"] = osum
                                # ---- single denominator matmul + reciprocal
                                if s >= 14 and (s - 14) % n_p == 0 and (s - 14) // n_p < n_qt:
                                    qi = (s - 14) // n_p
                                    ps_l = psA.tile(
                                        [1, 512], F32, tag="psA", name="ps_l"
                                    )
                                    nc.tensor.matmul(
                                        ps_l[:],
                                        ones_col[:],
                                        st[qi]["osum"][:],
                                        start=True,
                                        stop=True,
                                    )
                                    rlf = pB.tile([1, 512], F32, tag="rlf", name="rlf")
                                    nc.vector.reciprocal_approx_fast(
                                        out=rlf[:], in_=ps_l[:]
                                    )
                                    rl = pB.tile([1, 512], BF16, tag="rl", name="rl")
                                    nc.vector.tensor_copy(rl[:], rlf[:])
                                    pending.append(
                                        (st[qi]["ps_o"], rl, combT, qi, hl)
                                    )
                                if s % n_p == 0 and s > 0 and pending:
                                    flush_pending()
                            # drain the pipeline before the send DMA reads combT
                            while pending:
                                flush_pending()
                            # shards were streamed per q-tile by the flushes;
                            # redistribute head->token sharding in two halves so
                            # phase C can start on the first half early.
                            for hf in range(2):
                                nc.gpsimd.collective_compute(
                                    "AllToAll",
                                    mybir.AluOpType.bypass,
                                    replica_groups=[list(range(NC))],
                                    ins=[a2a_send[hl][hf][:]],
                                    outs=[a2a_recv[hl][hf][:]],
                                )

                    # ---------------- Phase C: out projection ----------------
                    # pB/psB/xtp closed above: their SBUF backs woutO below.
                    with (
                        tc.tile_pool(name="woutO", bufs=16) as woutO,
                        tc.tile_pool(name="psC", bufs=2, space="PSUM") as psC,
                    ):
                        # odd-head w_out tiles: fully resident (wait-free DMAs),
                        # streamed during phase C's even half / A2A window.
                        for g in range(4):
                            for e in range(4, 8):
                                wtile = woutO.tile(
                                    [128, 2, 512], BF16, tag="wout_o", name="wout_o"
                                )
                                nc.scalar.dma_start(
                                    out=wtile[:],
                                    in_=woutT.ap()[
                                        e * 256 : (e + 1) * 256,
                                        g * 512 : (g + 1) * 512,
                                    ].rearrange("(dc p) f -> p dc f", p=128),
                                )
                                weighth[(g, e)] = wtile

                        # combined^T input tiles per (contraction chunk, half).
                        # Emission in (head, half) dependency order: the DMA
                        # queue is in-order at runtime, so even-head tiles
                        # (both halves, gated on head 0's long-landed A2As)
                        # must not sit behind odd-head tiles that wait on the
                        # head-1 collectives.
                        comb = {}
                        for hi in range(2):
                            for hf in range(2):
                                for blk in range(8):
                                    cc = hi * 8 + blk
                                    ctile = pC.tile(
                                        [128, HTPC], BF16, tag="comb_in",
                                        name="comb_in", bufs=2 * DC,
                                    )
                                    nc.gpsimd.dma_start(
                                        out=ctile[:], in_=a2a_recv[hi][hf][blk]
                                    )
                                    comb[(cc, hf)] = ctile

                        def stat(cc, ts):
                            """[128,128] stationary: chunk cc, token sub-tile ts."""
                            return comb[(cc, ts // 2)][
                                :, (ts % 2) * 128 : (ts % 2 + 1) * 128
                            ]

                        # even-head partial sums first (head-0 A2A data, long
                        # landed), evicted to SBUF bf16.
                        partials = {}
                        for g in range(4):
                            for ts in range(TPC // 128):
                                psE = psC.tile([128, 512], F32, tag="psC")
                                for cc in range(8):
                                    nc.tensor.matmul(
                                        psE[:],
                                        stat(cc, ts),
                                        weighth[(g, cc // 2)][:, cc % 2, :],
                                        start=(cc == 0),
                                        stop=(cc == 7),
                                    )
                                pev = evC.tile(
                                    [128, 512], BF16, tag="pev", bufs=16, name="pev"
                                )
                                nc.scalar.copy(pev[:], psE[:])
                                partials[(g, ts)] = pev
                        # odd partials: token-half major so each group is gated
                        # only on the matching head-1 half-A2A.
                        for hf in range(2):
                            for g in range(4):
                                for ts in (2 * hf, 2 * hf + 1):
                                    psO = psC.tile([128, 512], F32, tag="psC")
                                    for cc in range(8, DC):
                                        nc.tensor.matmul(
                                            psO[:],
                                            stat(cc, ts),
                                            weighth[(g, cc // 2)][:, cc % 2, :],
                                            start=(cc == 8),
                                            stop=(cc == DC - 1),
                                        )
                                    ev = evC.tile([128, 512], F32, tag="ev")
                                    nc.vector.tensor_add(
                                        ev[:], psO[:], partials[(g, ts)][:]
                                    )
                                    nc.sync.dma_start(
                                        out=out_ext.ap()[
                                            ts * 128 : (ts + 1) * 128,
                                            g * 512 : (g + 1) * 512,
                                        ],
                                        in_=ev[:],
                                    )
    nc.finalize()
    return nc


def prep_inputs(x, w_qkv, w_out):
    """Host-side sharding. Returns list of per-core input dicts."""
    x = np.asarray(x, dtype=np.float32)
    w_qkv = np.asarray(w_qkv, dtype=np.float32)
    w_out = np.asarray(w_out, dtype=np.float32)

    xT = np.ascontiguousarray(x.reshape(T, D).T).astype(ml_dtypes.bfloat16)

    # w_out^T with rows permuted to (even heads | odd heads)
    woutT = w_out.T  # [cin, dout], cin = h*DH + d
    perm = [2 * i for i in range(8)] + [2 * i + 1 for i in range(8)]
    woutT_bf = np.ascontiguousarray(
        np.concatenate([woutT[h * DH : (h + 1) * DH] for h in perm], axis=0)
    ).astype(ml_dtypes.bfloat16)

    scale = np.float32(1.0 / np.sqrt(DH))
    ones = np.ones((128, 1), dtype=ml_dtypes.bfloat16)
    in_maps = []
    for c in range(NC):
        h0 = HL * c
        wq = w_qkv[h0 * DH : (h0 + HL) * DH] * scale  # [256, D]
        wk = w_qkv[H * DH + h0 * DH : H * DH + (h0 + HL) * DH]
        wv = w_qkv[2 * H * DH + h0 * DH : 2 * H * DH + (h0 + HL) * DH]
        wqkvT = np.ascontiguousarray(np.concatenate([wq, wk, wv], axis=0).T).astype(
            ml_dtypes.bfloat16
        )
        in_maps.append(
            {
                "xT": xT,
                "wqkvT": wqkvT,
                "woutT": woutT_bf,
                "ones_in": ones,
                "ones_row_in": np.ones((1, 128), dtype=ml_dtypes.bfloat16),
            }
        )
    return in_maps


def run(x, w_qkv, w_out, mm_dt=BF16, trace=False, tmpdir=None):
    key = str(mm_dt)
    if key not in _graph_cache:
        _graph_cache[key] = build_graph(mm_dt)
    nc = _graph_cache[key]
    in_maps = prep_inputs(x, w_qkv, w_out)
    res = run_bass_kernel_spmd(
        nc, in_maps, core_ids=list(range(NC)), trace=trace, tmpdir=tmpdir
    )
    out = np.concatenate([res.results[c]["out"] for c in range(NC)], axis=0)
    return out.reshape(B, S, D).astype(np.float32), res


def kernel(x, w_qkv, w_out):
    out, _ = run(x, w_qkv, w_out)
    return out
